# revision 20
# baseline (speedup 1.0000x reference)
"""Trainium2 Bass kernel for the HPNET loss (confidence + depth + rotation).

Contract: kernel(**inputs) takes the FULL unsharded inputs and returns the
full output (a tuple of three f32 scalars), distributing work across 8
NeuronCores internally.

Sharding (hardcoded): data-parallel over 8 cores.
  - confidence/confidence_gt/weight: batch dim 256 -> 32 batches per core,
    flattened per core to [128, 16384] each, quantized host-side to fp8
    (e4m3) and PACKED into one [128, 49152] tensor laid out chunk-by-chunk
    as [a_c0 | b_c0 | w_c0 | a_c1 | ...] so each chunk loads with a single
    DMA (10 stream DMAs total).
  - depth_and_rotation/ann_values/ann_flags: ROI dim 8192 -> 1024 per core,
    packed into one fp32 [128, 88] tensor (dr 40 | ann 40 | mask 8).
Outputs: depth/rotation per-partition partials [128, 2] plus the confidence
partial sums [1, 512] from PSUM; the tiny final reduction over cores
happens on host in float64.

Performance structure (measured ~60.7us on HW, from a 97.5us fp32
baseline; rel err ~1.2e-3 vs the 2e-2 gate):
  - The dominant constraint is HBM bandwidth SHARED between the two
    NeuronCores of a stack (~358 GB/s/core when all 8 cores stream in
    lockstep). Precision is the only lever on bytes: the confidence stream
    is read as fp8 (6.3 MB/core) and cast to bf16 in-flight by gpsimd
    SWDGE cast-DMAs (HWDGE cannot cast). The error this introduces
    (~1.2e-3 relative, dominated by fp8 quantization of a,b before the
    subtract) is 16x inside the tolerance, and it also removes the
    cross-core HBM contention that made fp32 runs vary 89-105us.
  - One cast-DMA per chunk on the gpsimd ring (Q7 descriptor emission
    ~3us per cast-DMA paces the stream; write side runs ~430 GB/s).
    io pool bufs=10 keeps every chunk resident so no DMA ever waits on a
    tile-ring slot.
  - Compute split across four engines: Vector does d=a-b and wd2=d2*w
    (both bf16 2x-mode, ~1.2us/2048-chunk); Scalar does the squares
    (ACT, in-place); TensorE reduces wd2 via ones-vector matmuls
    accumulating in PSUM (replacing the 1x-mode DVE accumulate, which
    was the bottleneck); host sums the [1,512] PSUM row.
  - Vector issue order is software-pipelined two chunks deep
    (sub_{k+2} ahead of mul_k) so scalar's square latency is hidden.
  - ROI math is vector-engine-only (squares as d*d multiplies; m_gt
    scaled by 1/|q|^2 via quat2mat homogeneity M(q/|q|) = M(q)/|q|^2,
    so no normalize-sqrt chain) except two tiny final sqrts on scalar,
    given two chunks of cross-engine slack. ROI stays fp32.
"""

import numpy as np
import ml_dtypes

_NCORES = 8
_B = 256
_HW = 256 * 256
_N = 8192
_PB = _B // _NCORES            # batches per core
_F = _PB * _HW // 128          # 16384 free elems per partition
_CHUNKS = (1024, 2048, 2048, 2048, 2048, 2048, 2048, 2048, 512, 512)
assert sum(_CHUNKS) == _F
_NCH = len(_CHUNKS)
_R = _N // _NCORES // 128      # 8 ROIs per partition
_OUTC = 2
_MAXCH = max(_CHUNKS)

_CACHE = {}


def build_nc():
    import concourse.bacc as bacc
    import concourse.mybir as mybir
    import concourse.tile as tile

    f32 = mybir.dt.float32
    bf16 = mybir.dt.bfloat16
    f8 = mybir.dt.float8e4
    Alu = mybir.AluOpType
    Act = mybir.ActivationFunctionType
    AxX = mybir.AxisListType.X

    nc = bacc.Bacc("TRN2", target_bir_lowering=False, debug=False,
                   num_devices=_NCORES)

    x = nc.dram_tensor("x", [128, 3 * _F], f8, kind="ExternalInput")
    roix = nc.dram_tensor("roix", [128, _R * 11], f32, kind="ExternalInput")
    out = nc.dram_tensor("out", [128, _OUTC], f32, kind="ExternalOutput")
    outc = nc.dram_tensor("outc", [1, 512], f32, kind="ExternalOutput")

    with tile.TileContext(nc) as tc:
        with tc.tile_pool(name="io", bufs=10) as io, \
                tc.tile_pool(name="wk", bufs=6) as wk, \
                tc.tile_pool(name="psum", bufs=1, space="PSUM") as psum, \
                tc.tile_pool(name="roi", bufs=1) as roi:

            accs = roi.tile([128, _OUTC], f32, tag="accs", name="accs")
            ones = roi.tile([128, 1], bf16, tag="ones", name="ones")
            nc.vector.memset(ones[:], 1.0)
            cps = psum.tile([1, 512], f32, tag="cps", name="cps")
            csb = roi.tile([1, 512], f32, tag="csb", name="csb")

            # ---- single ROI input load, first on the SP ring ----
            roit = roi.tile([128, _R * 11], f32, tag="roit", name="roit")
            nc.sync.dma_start(out=roit[:], in_=roix[:])

            dr3 = roit[:, 0:_R * 5].rearrange("p (r c) -> p r c", c=5)
            an3 = roit[:, _R * 5:_R * 10].rearrange("p (r c) -> p r c", c=5)
            mt = roit[:, _R * 10:_R * 11]

            # ROI persistent tiles
            W2 = 2 * _R
            Q = roi.tile([128, 4, W2], f32, tag="Q", name="Q")
            SQ = roi.tile([128, 4, W2], f32, tag="SQ", name="SQ")
            qd = roi.tile([128, 3, W2], f32, tag="qd", name="qd")
            pp = roi.tile([128, 6, W2], f32, tag="pp", name="pp")
            uv = roi.tile([128, 4, W2], f32, tag="uv", name="uv")
            M = roi.tile([128, 9, W2], f32, tag="M", name="M")
            d1 = roi.tile([128, 9, _R], f32, tag="d1", name="d1")
            f2 = roi.tile([128, 9, _R], f32, tag="f2", name="f2")
            d1s = roi.tile([128, 9, _R], f32, tag="d1s", name="d1s")
            f2s = roi.tile([128, 9, _R], f32, tag="f2s", name="f2s")
            dd = roi.tile([128, _R], f32, tag="dd", name="dd")
            dd2 = roi.tile([128, _R], f32, tag="dd2", name="dd2")
            nrm2 = roi.tile([128, _R], f32, tag="nrm2", name="nrm2")
            rinv2 = roi.tile([128, _R], f32, tag="rinv2", name="rinv2")
            n1sq = roi.tile([128, _R], f32, tag="n1sq", name="n1sq")
            n2sq = roi.tile([128, _R], f32, tag="n2sq", name="n2sq")
            n1 = roi.tile([128, _R], f32, tag="n1", name="n1")
            n2 = roi.tile([128, _R], f32, tag="n2", name="n2")
            nmin = roi.tile([128, _R], f32, tag="nmin", name="nmin")
            dscr = roi.tile([128, _R], f32, tag="dscr", name="dscr")
            rscr = roi.tile([128, _R], f32, tag="rscr", name="rscr")

            def roi_vec():
                """Vector-only ROI chain through the Frobenius sums."""
                # raw quats into Q: cols 0..R-1 = pred (dr), R..2R-1 = ann
                nc.vector.tensor_copy(
                    Q[:, :, 0:_R], dr3[:, :, 1:5].rearrange("p r c -> p c r"))
                nc.vector.tensor_copy(
                    Q[:, :, _R:W2], an3[:, :, 1:5].rearrange("p r c -> p c r"))
                # depth loss
                nc.vector.tensor_sub(dd[:], dr3[:, :, 0], an3[:, :, 0])
                nc.vector.tensor_mul(dd2[:], dd[:], dd[:])
                nc.vector.scalar_tensor_tensor(
                    out=dscr[:], in0=dd2[:], scalar=1.0, in1=mt,
                    op0=Alu.mult, op1=Alu.mult,
                    accum_out=accs[:, 0:1])
                # quat squares, |q_pred|^2, 1/|q|^2
                nc.vector.tensor_mul(SQ[:], Q[:], Q[:])
                nc.vector.tensor_reduce(
                    out=nrm2[:], in_=SQ[:, :, 0:_R].rearrange("p c r -> p r c"),
                    axis=AxX, op=Alu.add)
                nc.vector.reciprocal(rinv2[:], nrm2[:])
                # quat2mat building blocks for both quats (raw, unnormalized)
                nc.vector.tensor_scalar_mul(qd[:], Q[:, 0:3, :], 2.0)
                pairs = [(0, 1), (0, 2), (0, 3), (1, 2), (1, 3), (2, 3)]
                for k, (xq, yq) in enumerate(pairs):
                    nc.vector.tensor_mul(pp[:, k, :], qd[:, xq, :],
                                         Q[:, yq, :])
                nc.vector.tensor_sub(uv[:, 0, :], SQ[:, 0, :], SQ[:, 3, :])
                nc.vector.tensor_sub(uv[:, 1, :], SQ[:, 1, :], SQ[:, 2, :])
                nc.vector.tensor_add(uv[:, 2, :], SQ[:, 0, :], SQ[:, 3, :])
                nc.vector.tensor_add(uv[:, 3, :], SQ[:, 1, :], SQ[:, 2, :])
                # matrix entries, column-major e = 3*col+row
                P01, P02, P03, P12, P13, P23 = (pp[:, k, :] for k in range(6))
                u, v, u2, v2 = (uv[:, k, :] for k in range(4))
                nc.vector.tensor_add(M[:, 0, :], u, v)       # m00
                nc.vector.tensor_add(M[:, 1, :], P12, P03)   # m10
                nc.vector.tensor_sub(M[:, 2, :], P13, P02)   # m20
                nc.vector.tensor_sub(M[:, 3, :], P12, P03)   # m01
                nc.vector.tensor_sub(M[:, 4, :], u, v)       # m11
                nc.vector.tensor_add(M[:, 5, :], P23, P01)   # m21
                nc.vector.tensor_add(M[:, 6, :], P13, P02)   # m02
                nc.vector.tensor_sub(M[:, 7, :], P23, P01)   # m12
                nc.vector.tensor_sub(M[:, 8, :], u2, v2)     # m22
                # m_gt = M(q_pred)/|q|^2  (homogeneity of quat2mat)
                for e in range(9):
                    nc.vector.tensor_mul(M[:, e, 0:_R], M[:, e, 0:_R],
                                         rinv2[:])
                # diffs: RY flips sign of pred columns 0 and 2
                mg = M[:, :, 0:_R]
                mp = M[:, :, _R:W2]
                nc.vector.tensor_sub(d1[:], mg, mp)
                nc.vector.tensor_add(f2[:, 0:3, :], mg[:, 0:3, :],
                                     mp[:, 0:3, :])
                nc.vector.tensor_copy(f2[:, 3:6, :], d1[:, 3:6, :])
                nc.vector.tensor_add(f2[:, 6:9, :], mg[:, 6:9, :],
                                     mp[:, 6:9, :])
                nc.vector.tensor_mul(d1s[:], d1[:], d1[:])
                nc.vector.tensor_mul(f2s[:], f2[:], f2[:])
                nc.vector.tensor_reduce(
                    out=n1sq[:], in_=d1s.rearrange("p e r -> p r e"),
                    axis=AxX, op=Alu.add)
                nc.vector.tensor_reduce(
                    out=n2sq[:], in_=f2s.rearrange("p e r -> p r e"),
                    axis=AxX, op=Alu.add)

            def roi_act():
                nc.scalar.activation(n1[:], n1sq[:], Act.Sqrt)
                nc.scalar.activation(n2[:], n2sq[:], Act.Sqrt)

            def roi_fin():
                nc.vector.tensor_tensor(nmin[:], n1[:], n2[:], op=Alu.min)
                nc.vector.scalar_tensor_tensor(
                    out=rscr[:], in0=nmin[:], scalar=1.0, in1=mt,
                    op0=Alu.mult, op1=Alu.mult,
                    accum_out=accs[:, 1:2])

            # ---------------- confidence loss stream ----------------
            offs = []
            off = 0
            for ch in _CHUNKS:
                offs.append(off)
                off += 3 * ch

            xts, ds = {}, {}

            def emit_dma(k):
                ch = _CHUNKS[k]
                xt = io.tile([128, 3 * _MAXCH], bf16, tag="xt", name="xt")
                sl = slice(offs[k], offs[k] + 3 * ch)
                nc.gpsimd.dma_start(out=xt[:, 0:3 * ch], in_=x[:, sl])
                xts[k] = xt

            def emit_sub(k):
                ch = _CHUNKS[k]
                d = wk.tile([128, _MAXCH], bf16, tag="d", name="d")
                nc.vector.tensor_sub(d[:, 0:ch], xts[k][:, 0:ch],
                                     xts[k][:, ch:2 * ch])
                ds[k] = d

            def emit_sq(k):
                ch = _CHUNKS[k]
                if k >= _NCH - 2:
                    # tail chunks: square on vector (bf16 2x, no cross-
                    # engine hop in the post-DMA critical chain)
                    nc.vector.tensor_mul(ds[k][:, 0:ch], ds[k][:, 0:ch],
                                         ds[k][:, 0:ch])
                else:
                    nc.scalar.activation(ds[k][:, 0:ch], ds[k][:, 0:ch],
                                         Act.Square)

            def emit_stt(k):
                ch = _CHUNKS[k]
                nc.vector.tensor_mul(ds[k][:, 0:ch], ds[k][:, 0:ch],
                                     xts[k][:, 2 * ch:3 * ch])
                for i in range(0, ch, 512):
                    w512 = min(512, ch - i)
                    nc.tensor.matmul(
                        cps[:, 0:w512], ones[:], ds[k][:, i:i + w512],
                        start=(k == 0 and i == 0),
                        stop=(k == _NCH - 1 and i + 512 >= ch))

            n = _NCH
            for k in range(n):
                emit_dma(k)
            emit_sub(0)
            roi_vec()
            emit_sub(1)
            for k in range(2, n):
                emit_sub(k)
                if k == 3:
                    roi_act()
                emit_sq(k - 2)
                emit_stt(k - 2)
                if k == 5:
                    roi_fin()
            emit_sq(n - 2)
            emit_stt(n - 2)
            emit_sq(n - 1)
            emit_stt(n - 1)
            nc.vector.tensor_copy(csb[:], cps[:])

            nc.sync.dma_start(out=out[:], in_=accs[:])
            nc.sync.dma_start(out=outc[:], in_=csb[:])

    nc.compile()
    return nc


def _get_nc():
    if "nc" not in _CACHE:
        _CACHE["nc"] = build_nc()
    return _CACHE["nc"]


def make_in_maps(confidence, confidence_gt, weight, depth_and_rotation,
                 ann_values, ann_flags):
    a = np.ascontiguousarray(confidence, dtype=np.float32).reshape(
        _NCORES, 128, _F)
    b = np.ascontiguousarray(confidence_gt, dtype=np.float32).reshape(
        _NCORES, 128, _F)
    w = np.ascontiguousarray(weight, dtype=np.float32).reshape(
        _NCORES, 128, _F)
    # pack chunk-by-chunk: [a_ck | b_ck | w_ck] ...
    parts = []
    off = 0
    for ch in _CHUNKS:
        parts += [a[:, :, off:off + ch], b[:, :, off:off + ch],
                  w[:, :, off:off + ch]]
        off += ch
    xp = np.concatenate(parts, axis=2).astype(ml_dtypes.float8_e4m3)
    dr = np.ascontiguousarray(depth_and_rotation, dtype=np.float32).reshape(
        _NCORES, 128, _R * 5)
    an = np.ascontiguousarray(ann_values, dtype=np.float32).reshape(
        _NCORES, 128, _R * 5)
    mk = np.ascontiguousarray(ann_flags).astype(np.float32).reshape(
        _NCORES, 128, _R)
    rx = np.concatenate([dr, an, mk], axis=2)  # [NCORES, 128, 88]
    return [dict(x=xp[c], roix=rx[c]) for c in range(_NCORES)]


def reduce_outs(outs):
    """outs: list of per-core {'out': [128, _OUTC]} -> (conf, depth, rot)."""
    P = np.stack([o["out"] for o in outs]).astype(np.float64)
    C = np.stack([o["outc"] for o in outs]).astype(np.float64)
    conf = C.sum() / float(_HW)
    dep = P[:, :, 0].sum() / float(_N)
    rot = P[:, :, 1].sum() / float(_N)
    return (np.float32(conf), np.float32(dep), np.float32(rot))


def kernel(confidence, confidence_gt, weight, depth_and_rotation,
           ann_values, ann_flags):
    from concourse.bass_utils import run_bass_kernel_spmd
    nc = _get_nc()
    in_maps = make_in_maps(confidence, confidence_gt, weight,
                           depth_and_rotation, ann_values, ann_flags)
    res = run_bass_kernel_spmd(nc, in_maps, core_ids=list(range(_NCORES)))
    return reduce_outs(res.results)
